# revision 43
# baseline (speedup 1.0000x reference)
"""Multi-head attention (B=4, S=1500, D=1024, H=16) on 8 TRN2 NeuronCores.

Sharding: (batch, head-half) -> core c = 2*b + h; each core computes the
full attention for batch b, heads h*8..h*8+7, plus its partial contribution
to the output projection (contraction over its 512 features). Host sums the
two partials per batch and stacks.

All matmul operands are bf16 (measured end-to-end rel err ~5e-3 vs the 2e-2
gate). Per-core layout / schedule:
  xT   [1024,1500]  host-pretransposed x[b].T (bf16)
  qT/kT = W^T.T @ xT -> feature-major [4x128, 1500/1536] per 128-feat tile
  v    [1500, 8*65] natural, 65th column per head = ones (softmax denom);
       bv folded into v (softmax rows sum to 1 -> wv + bv comes out).
  scores: per (head-pair hp, head e, 2-kt round r): S^T tiles [128k, q<=512]
       into a double-buffered PSUM tile [128,1024]; exp on ACT per round
       (strided APs skip the pad columns of the ragged 476-wide q chunk).
  U-trick: wv_q[q(128),65] += P^T[k,q].T @ v_aug[k,65] per (qt, kt) -- the
       free dim is 65 instead of ~500, saving ~70k PE cycles/core vs the
       [65,q] orientation; denominators ride along as column 64. One
       start/stop per u tile: matmul start=True zeroes the whole 2KB PSUM
       zero-region, so per-qt groups would wipe each other.
  normalize: per (hp,e) one DVE reciprocal [128,4] + one stride-0-broadcast
       multiply (q-major: the denominator is a per-partition scalar, so the
       baseline's DRAM broadcast bounce disappears).
  wv_q -> wvT via 4 accumulating PE transposes into one [128,512] PSUM tile
       + one DVE copy per (hp, chunk), then the output projection
       y_partial[1500,1024] = wvT.T @ woT (y stored bf16; host upcasts).

The emission order software-pipelines everything around the in-order PE
stream: a lead-in runs six st-interleaved projection pieces under the x
DMA, then v; attention rounds carry U-work lagged two rounds ahead of the
scores (so nothing waits on a just-issued exp), transposes deferred into
the next hp, and the remaining projection/output-projection pieces woven
in as fillers at the known stall sites (r3 + the two hp-boundary slots).
PSUM budget: st 2x2 + u 2 + pj 2 = 8 banks. PE busy ~176us of ~211us
total; ACT (exp) ~147us.
"""

import os
import numpy as np

N_STATE = 1024
B = 4
S = 1500
F = 512          # features per core (8 heads x 64)
NST = 8          # state k-tiles of 128 (contraction for projections)
NKT = 12         # seq k-tiles of 128 (attention contraction), last = 92
KPAD = 1536      # padded k extent (12*128)
QCH = [(0, 512), (512, 512), (1024, 476)]  # q chunks
VBLK = 520       # 8 heads * 65 cols (64 d + ones) per seq tile in v_sb
SCALE = 0.125    # 1/sqrt(64)
NCORES = 8
NR = 6           # kt rounds per (hp, e, chunk): rounds of 2 kt

_CACHE = {}
LAST_RESULTS = None


def _build():
    import concourse.bass as bass
    import concourse.mybir as mybir
    import concourse.tile as tile
    from concourse import bacc

    f32 = mybir.dt.float32
    bf16 = mybir.dt.bfloat16
    Exp = mybir.ActivationFunctionType.Exp
    Copy = mybir.ActivationFunctionType.Copy
    mdt = bf16

    nc = bacc.Bacc("TRN2", target_bir_lowering=False, debug=False,
                   num_devices=NCORES)

    xT = nc.dram_tensor("xT", [N_STATE, S], mdt, kind="ExternalInput").ap()
    wq = nc.dram_tensor("wq", [F, N_STATE], mdt, kind="ExternalInput").ap()
    wk = nc.dram_tensor("wk", [F, N_STATE], mdt, kind="ExternalInput").ap()
    wv = nc.dram_tensor("wv", [128, NST * F], mdt, kind="ExternalInput").ap()
    wo = nc.dram_tensor("wo", [128, 4 * N_STATE], mdt,
                        kind="ExternalInput").ap()
    bq = nc.dram_tensor("bq", [F], f32, kind="ExternalInput").ap()
    bv = nc.dram_tensor("bv", [F], f32, kind="ExternalInput").ap()
    zinit = nc.dram_tensor("zinit", [4 * (KPAD - S)], mdt,
                           kind="ExternalInput").ap()
    ident = nc.dram_tensor("ident", [128, 128], mdt,
                           kind="ExternalInput").ap()
    y = nc.dram_tensor("y", [S, N_STATE], mdt, kind="ExternalOutput").ap()

    def mm(out, lhsT, rhs, **kw):
        nc.tensor.matmul(out=out, lhsT=lhsT, rhs=rhs, **kw)

    with tile.TileContext(nc) as tc:
        with (
            tc.tile_pool(name="sb", bufs=1) as sb,
            tc.tile_pool(name="sbw", bufs=6) as sbw,
            tc.tile_pool(name="ptp", bufs=20) as ptp,
            tc.tile_pool(name="wvq", bufs=4) as wvqp,
            tc.tile_pool(name="sm", bufs=8) as sm,
            tc.tile_pool(name="ysp", bufs=12) as ysp,
            tc.tile_pool(name="pst", bufs=2, space="PSUM") as pstp,
            tc.tile_pool(name="pu", bufs=2, space="PSUM") as pup,
            tc.tile_pool(name="pj", bufs=2, space="PSUM") as pjp,
        ):
            # ---------------- persistent SBUF ----------------
            xT_sb = sb.tile([128, NST * S], mdt, name="xT_sb", tag="xT")
            qT_sb = sb.tile([128, 4 * S], mdt, name="qT_sb", tag="qT")
            kT_sb = sb.tile([128, 4 * KPAD], mdt, name="kT_sb", tag="kT")
            v_sb = sb.tile([128, NKT * VBLK], mdt, name="v_sb", tag="v")
            wvT_sb = sb.tile([128, 4 * KPAD], mdt, name="wvT_sb", tag="wvT")
            wv_sb = sb.tile([128, NST * F], mdt, name="wv_sb", tag="wvw")
            wo_sb = sb.tile([128, 4 * N_STATE], mdt, name="wo_sb", tag="wow")
            bq_sb = sb.tile([128, 4], f32, name="bq_sb", tag="bq")
            bvb_sb = sb.tile([128, F], f32, name="bvb_sb", tag="bvb")
            id_sb = sb.tile([128, 128], mdt, name="id_sb", tag="id")
            wqsl = [sb.tile([128, NST * 128], mdt, name=f"wqsl{ft}",
                            tag=f"wq{ft}") for ft in range(4)]

            # ---------------- input DMAs (ordered by first use) -----------
            wsl_k0 = sbw.tile([128, NST * 128], mdt, name="wsl_k0",
                              tag="wsl")
            nc.sync.dma_start(out=wsl_k0[:, 0:256], in_=wk[0:128, 0:256])
            nc.sync.dma_start(out=bq_sb[:],
                              in_=bq.rearrange("(f p) -> p f", p=128))
            nc.sync.dma_start(out=wsl_k0[:, 256:1024],
                              in_=wk[0:128, 256:1024])
            for st in range(NST):
                nc.sync.dma_start(
                    out=xT_sb[:, st * S:(st + 1) * S],
                    in_=xT[st * 128:(st + 1) * 128, :])
            nc.sync.dma_start(out=wqsl[0][:], in_=wq[0:128, :])
            wsl_k1 = sbw.tile([128, NST * 128], mdt, name="wsl_k1",
                              tag="wsl")
            nc.sync.dma_start(out=wsl_k1[:], in_=wk[128:256, :])
            nc.sync.dma_start(out=wqsl[1][:], in_=wq[128:256, :])
            nc.sync.dma_start(out=wv_sb[:], in_=wv)
            nc.sync.dma_start(
                out=bvb_sb[:], in_=bv[None, :].to_broadcast((128, F)))
            nc.vector.memset(
                v_sb[:].rearrange("p (t h c) -> p t h c",
                                  t=NKT, h=8)[:, :, :, 64:65], 1.0)
            nc.sync.dma_start(out=id_sb[:], in_=ident)
            nc.sync.dma_start(
                out=kT_sb[:].rearrange("p (f c) -> p f c",
                                       f=4)[:, :, S:KPAD],
                in_=zinit.rearrange("(f c) -> f c", f=4)[None].to_broadcast(
                    (128, 4, KPAD - S)))

            # ---------------- projections ----------------
            def proj_piece(wsl, dst, dstride, biased, ft, qi):
                """One (feature-tile, q-chunk) slice of the q/k projection."""
                q0, qn = QCH[qi]
                pacc = pjp.tile([128, 512], f32, name="pacc", tag="pj")
                for st in range(NST):
                    mm(out=pacc[:, 0:qn],
                       lhsT=wsl[:, st * 128:(st + 1) * 128],
                       rhs=xT_sb[:, st * S + q0:st * S + q0 + qn],
                       start=(st == 0), stop=(st == NST - 1))
                if biased:
                    nc.vector.tensor_scalar_add(
                        out=dst[:, ft * dstride + q0:ft * dstride + q0 + qn],
                        in0=pacc[:, 0:qn], scalar1=bq_sb[:, ft:ft + 1])
                else:
                    nc.vector.tensor_copy(
                        out=dst[:, ft * dstride + q0:ft * dstride + q0 + qn],
                        in_=pacc[:, 0:qn])

            def k_slice(ft):
                wsl = sbw.tile([128, NST * 128], mdt, name="wsl", tag="wsl")
                nc.sync.dma_start(out=wsl[:],
                                  in_=wk[ft * 128:(ft + 1) * 128, :])
                return wsl

            def v_piece(sq):
                """v projection for one seq tile, bv folded in."""
                sn = min(128, S - sq * 128)
                pv = pjp.tile([128, 512], f32, name="pv", tag="pj")
                for st in range(NST):
                    mm(out=pv[0:sn, :],
                       lhsT=xT_sb[:, st * S + sq * 128:
                                  st * S + sq * 128 + sn],
                       rhs=wv_sb[:, st * F:(st + 1) * F],
                       start=(st == 0), stop=(st == NST - 1))
                nc.vector.tensor_add(
                    out=v_sb[0:sn, sq * VBLK:(sq + 1) * VBLK].rearrange(
                        "p (h c) -> p h c", h=8)[:, :, 0:64],
                    in0=pv[0:sn, :].rearrange("p (h c) -> p h c", h=8),
                    in1=bvb_sb[0:sn, :].rearrange("p (h c) -> p h c", h=8))

            def out_piece(sq, ch, quarters=False):
                """Half of the output projection for one seq tile."""
                sn = min(128, S - sq * 128)
                py = pjp.tile([128, 512], f32, name="py", tag="pj")
                nq = 2 if quarters else 1
                for sub in range(nq):
                    c0 = ch * 512 + sub * 256
                    w = 512 // nq
                    for hp in range(4):
                        mm(out=py[0:sn, sub * w:(sub + 1) * w],
                           lhsT=wvT_sb[:, hp * KPAD + sq * 128:
                                       hp * KPAD + sq * 128 + sn],
                           rhs=wo_sb[:, hp * N_STATE + c0:
                                     hp * N_STATE + c0 + w],
                           start=(hp == 0 and sub == 0),
                           stop=(hp == 3 and sub == nq - 1))
                    yt = ysp.tile([128, w], mdt, name="yt", tag="yt")
                    use_act = (quarters and sub == 1) or (sq >= 8 and ch == 1)
                    if use_act:
                        nc.scalar.activation(yt[0:sn, :],
                                             py[0:sn, sub * w:(sub + 1) * w],
                                             Copy)
                    else:
                        nc.vector.tensor_copy(
                            out=yt[0:sn, :],
                            in_=py[0:sn, sub * w:(sub + 1) * w])
                    nc.sync.dma_start(
                        out=y[sq * 128:sq * 128 + sn, c0:c0 + w],
                        in_=yt[0:sn, :])

            # ---------------- filler management ----------------
            fillers = []

            def fill(n):
                for _ in range(min(n, len(fillers))):
                    fillers.pop(0)()

            # ---------------- attention ----------------
            def scores(hp, e, qi, r):
                q0, qn = QCH[qi]
                pb = 64 * e
                st_ps = pstp.tile([128, 1024], f32, name="st_ps", tag="st")
                for j in range(2):
                    kt = 2 * r + j
                    mm(out=st_ps[:, j * 512:j * 512 + qn],
                       lhsT=kT_sb[pb:pb + 64,
                                  hp * KPAD + kt * 128:
                                  hp * KPAD + (kt + 1) * 128],
                       rhs=qT_sb[pb:pb + 64,
                                 hp * S + q0:hp * S + q0 + qn])
                pt = ptp.tile([128, 1024], mdt, name="pt", tag="pt")
                if qn == 512:
                    nc.scalar.activation(pt[:], st_ps[:], Exp, scale=SCALE)
                else:
                    nc.scalar.activation(
                        pt[:].rearrange("p (j q) -> p j q",
                                        j=2)[:, :, 0:qn],
                        st_ps[:].rearrange("p (j q) -> p j q",
                                           j=2)[:, :, 0:qn],
                        Exp, scale=SCALE)
                return pt

            def u_round(hp, e, r, pt, u):
                head = 2 * hp + e
                # start=True zeroes the whole 2KB psum zero-region (bank),
                # so exactly one start/stop for the tile's four qt groups.
                for qt in range(4):
                    for j in range(2):
                        kt = 2 * r + j
                        kn = min(128, S - kt * 128)
                        mm(out=u[:, qt * 65:qt * 65 + 65],
                           lhsT=pt[0:kn, j * 512 + qt * 128:
                                   j * 512 + (qt + 1) * 128],
                           rhs=v_sb[0:kn, kt * VBLK + head * 65:
                                    kt * VBLK + head * 65 + 65],
                           start=(r == 0 and j == 0 and qt == 0),
                           stop=(r == NR - 1 and j == 1 and qt == 3))

            def normalize(hp, e, u, wv_q):
                rc = sm.tile([128, 4], f32, name="rc", tag="rc")
                nc.vector.reciprocal(rc[:], u[:, 64:260:65])
                nc.vector.tensor_mul(
                    wv_q[:].rearrange("p (q d) -> p q d",
                                      q=4)[:, :, (2 * hp + e) * 64:
                                           (2 * hp + e) * 64 + 64],
                    u[:, 0:260].rearrange("p (q c) -> p q c",
                                          q=4)[:, :, 0:64],
                    rc[:].unsqueeze(2).to_broadcast((128, 4, 64)))

            def transposes(hp, qi, wv_q):
                q0, _ = QCH[qi]
                tp = pjp.tile([128, 512], mdt, name="tp", tag="pj")
                for qt in range(4):
                    nc.tensor.matmul(
                        out=tp[:, qt * 128:(qt + 1) * 128],
                        lhsT=wv_q[:, qt * 512 + hp * 128:
                                  qt * 512 + (hp + 1) * 128],
                        rhs=id_sb[:], is_transpose=True,
                        start=(qt == 0), stop=(qt == 3))
                nc.vector.tensor_copy(
                    out=wvT_sb[:, hp * KPAD + q0:hp * KPAD + q0 + 512],
                    in_=tp[:])

            pending = []
            deferred = []

            def flush_pending():
                while pending:
                    h, q, w = pending.pop(0)
                    transposes(h, q, w)

            # ---------------- emission schedule ----------------
            # lead-in: kT(ft0) + qT(ft0, chunk0) + all of v, so hp0/chunk0
            # attention can start and its U rounds have every v tile. The
            # four ft0 pieces run st-interleaved so each arriving x tile
            # feeds ~0.9us of PE work instead of 0.2us.
            lead = [(wsl_k0, kT_sb, KPAD, False, 0, 0),
                    (wqsl[0], qT_sb, S, True, 0, 0),
                    (wsl_k0, kT_sb, KPAD, False, 0, 1),
                    (wsl_k0, kT_sb, KPAD, False, 0, 2),
                    (wsl_k1, kT_sb, KPAD, False, 1, 0),
                    (wqsl[1], qT_sb, S, True, 1, 0)]
            pools = ["pj", "pj", "st", "st", "u", "u"]
            lead_acc = []
            for gi, (wsl, dst, dstride, biased, ft, qi) in enumerate(lead):
                pool = {"pj": pjp, "st": pstp, "u": pup}[pools[gi]]
                lead_acc.append(pool.tile(
                    [128, 512], f32, name=f"lacc{gi}", tag=pools[gi]))
            for st in range(NST):
                for gi, (wsl, dst, dstride, biased, ft, qi) in \
                        enumerate(lead):
                    q0, qn = QCH[qi]
                    mm(out=lead_acc[gi][:, 0:qn],
                       lhsT=wsl[:, st * 128:(st + 1) * 128],
                       rhs=xT_sb[:, st * S + q0:st * S + q0 + qn],
                       start=(st == 0), stop=(st == NST - 1))
            # ACT is idle before attention: alternate the lead copies
            # across ACT/DVE so they drain in parallel (first scores gate
            # on the kc0/qc0 copies).
            Ident = mybir.ActivationFunctionType.Identity
            for gi, (wsl, dst, dstride, biased, ft, qi) in enumerate(lead):
                q0, qn = QCH[qi]
                dsl = dst[:, ft * dstride + q0:ft * dstride + q0 + qn]
                if gi % 2 == 0:
                    if biased:
                        nc.scalar.activation(dsl, lead_acc[gi][:, 0:qn],
                                             Ident,
                                             bias=bq_sb[:, ft:ft + 1])
                    else:
                        nc.scalar.activation(dsl, lead_acc[gi][:, 0:qn],
                                             Copy)
                elif biased:
                    nc.vector.tensor_scalar_add(
                        out=dsl, in0=lead_acc[gi][:, 0:qn],
                        scalar1=bq_sb[:, ft:ft + 1])
                else:
                    nc.vector.tensor_copy(out=dsl,
                                          in_=lead_acc[gi][:, 0:qn])
            for sq in range(NKT):
                v_piece(sq)

            # filler queue: remaining projections in deadline order
            # (kT ft before attention hp=ft; qT chunk qi before chunk qi).
            for qi in range(1, 3):
                fillers.append(
                    lambda qi=qi: proj_piece(wsl_k1, kT_sb, KPAD, False,
                                             1, qi))
            for ft in range(2, 4):
                wsl_k = k_slice(ft)
                nc.sync.dma_start(out=wqsl[ft][:],
                                  in_=wq[ft * 128:(ft + 1) * 128, :])
                for qi in range(3):
                    fillers.append(
                        lambda ft=ft, qi=qi, w=wsl_k: proj_piece(
                            w, kT_sb, KPAD, False, ft, qi))
                fillers.append(
                    lambda ft=ft: proj_piece(wqsl[ft], qT_sb, S, True,
                                             ft, 0))
            nc.sync.dma_start(out=wo_sb[:], in_=wo)
            for qi in range(1, 3):
                for ft in range(4):
                    fillers.append(
                        lambda ft=ft, qi=qi: proj_piece(
                            wqsl[ft], qT_sb, S, True, ft, qi))

            for qi in range(3):
                wv_q = wvqp.tile([128, 4 * 512], mdt, name="wv_q",
                                 tag="wvq")
                for hp in range(4):
                    us = [pup.tile([128, 260], f32, name=f"u{e}", tag="u")
                          for e in range(2)]
                    pts = {}
                    # U rounds lag the scores by TWO rounds and sit BEFORE
                    # them in the in-order PE stream, so the PE never waits
                    # on an exp that was just issued.
                    for r in range(NR):
                        if r >= 2:
                            u_round(hp, 0, r - 2, pts.pop((0, r - 2)),
                                    us[0])
                            u_round(hp, 1, r - 2, pts.pop((1, r - 2)),
                                    us[1])
                        if r == 2:
                            flush_pending()
                        if r == 4:
                            fill(1)
                        pts[(0, r)] = scores(hp, 0, qi, r)
                        pts[(1, r)] = scores(hp, 1, qi, r)
                    u_round(hp, 0, NR - 2, pts.pop((0, NR - 2)), us[0])
                    u_round(hp, 1, NR - 2, pts.pop((1, NR - 2)), us[1])
                    fill(1)
                    u_round(hp, 0, NR - 1, pts.pop((0, NR - 1)), us[0])
                    normalize(hp, 0, us[0], wv_q)
                    u_round(hp, 1, NR - 1, pts.pop((1, NR - 1)), us[1])
                    normalize(hp, 1, us[1], wv_q)
                    if qi == 2 and hp == 3:
                        # last hp: transposes must precede the tail
                        # out-pieces the boundary fills are about to pop
                        pending.append((hp, qi, wv_q))
                        flush_pending()
                        fill(1)
                        fill(1)
                    else:
                        fill(1)
                        fill(1)
                        pending.append((hp, qi, wv_q))
                # out-projection for this chunk becomes filler work for
                # the later chunks' ACT-bound stretches; half of chunk0's
                # pieces are deferred one more chunk to feed chunk2
                for sq in range(4 * qi, 4 * qi + 4):
                    fillers.append(
                        lambda sq=sq: out_piece(sq, 0))
                for sq in range(4 * qi, 4 * qi + 4):
                    item = (lambda sq=sq: out_piece(sq, 1))
                    if qi == 0:
                        deferred.append(item)
                    else:
                        fillers.append(item)
                fillers.extend(deferred)
                deferred = []
            flush_pending()
            while fillers:
                fillers.pop(0)()

    nc.compile()
    return nc


def get_nc():
    if "nc" not in _CACHE:
        _CACHE["nc"] = _build()
    return _CACHE["nc"]


def make_in_maps(x, Wq, bq, Wk, Wv, bv, Wo, bo):
    import ml_dtypes
    bf = ml_dtypes.bfloat16
    x = np.asarray(x, dtype=np.float32)
    Wq = np.asarray(Wq, dtype=np.float32)
    Wk = np.asarray(Wk, dtype=np.float32)
    Wv = np.asarray(Wv, dtype=np.float32)
    Wo = np.asarray(Wo, dtype=np.float32)
    bq = np.asarray(bq, dtype=np.float32)
    bv = np.asarray(bv, dtype=np.float32)
    ident = np.eye(128, dtype=bf)
    in_maps = []
    for c in range(NCORES):
        b, h = divmod(c, 2)
        sl = slice(h * F, (h + 1) * F)
        wq_t = Wq[sl, :].T.reshape(8, 128, 4, 128)
        wk_t = Wk[sl, :].T.reshape(8, 128, 4, 128)
        wv_t = Wv[sl, :].T.reshape(8, 128, 512)
        wo_t = Wo[:, sl].T.reshape(4, 128, 1024)
        in_maps.append(dict(
            xT=np.ascontiguousarray(x[b].T).astype(bf),
            wq=np.ascontiguousarray(
                wq_t.transpose(2, 1, 0, 3).reshape(512, 1024)).astype(bf),
            wk=np.ascontiguousarray(
                wk_t.transpose(2, 1, 0, 3).reshape(512, 1024)).astype(bf),
            wv=np.ascontiguousarray(
                wv_t.transpose(1, 0, 2).reshape(128, 4096)).astype(bf),
            wo=np.ascontiguousarray(
                wo_t.transpose(1, 0, 2).reshape(128, 4096)).astype(bf),
            bq=np.ascontiguousarray(bq[sl]),
            bv=np.ascontiguousarray(bv[sl]),
            zinit=np.zeros(4 * (KPAD - S), dtype=bf),
            ident=ident,
        ))
    return in_maps


def kernel(x, Wq, bq, Wk, Wv, bv, Wo, bo):
    global LAST_RESULTS
    from concourse.bass_utils import run_bass_kernel_spmd

    # An ambient BASS_TRACE=1 makes run_bass_kernel_spmd fetch the axon NTFF
    # profile hook; in containers where antenv.axon_hooks is absent that
    # import crashes. Disable tracing for this call when the hook is missing.
    try:
        import antenv.axon_hooks  # noqa: F401
    except ImportError:
        os.environ["BASS_NEVER_TRACE"] = "1"

    nc = get_nc()
    in_maps = make_in_maps(x, Wq, bq, Wk, Wv, bv, Wo, bo)
    res = run_bass_kernel_spmd(nc, in_maps, list(range(NCORES)))
    LAST_RESULTS = res
    bo32 = np.asarray(bo, dtype=np.float32)
    out = np.stack([res.results[2 * b]["y"].astype(np.float32)
                    + res.results[2 * b + 1]["y"].astype(np.float32)
                    + bo32[None, :] for b in range(B)])
    return out.astype(np.float32)


# revision 44
# speedup vs baseline: 1.0031x; 1.0031x over previous
"""Multi-head attention (B=4, S=1500, D=1024, H=16) on 8 TRN2 NeuronCores.

Sharding: (batch, head-half) -> core c = 2*b + h; each core computes the
full attention for batch b, heads h*8..h*8+7, plus its partial contribution
to the output projection (contraction over its 512 features). Host sums the
two partials per batch and stacks.

All matmul operands are bf16 (measured end-to-end rel err ~5e-3 vs the 2e-2
gate). Per-core layout / schedule:
  xT   [1024,1500]  host-pretransposed x[b].T (bf16)
  qT/kT = W^T.T @ xT -> feature-major [4x128, 1500/1536] per 128-feat tile
  v    [1500, 8*65] natural, 65th column per head = ones (softmax denom);
       bv folded into v (softmax rows sum to 1 -> wv + bv comes out).
  scores: per (head-pair hp, head e, 2-kt round r): S^T tiles [128k, q<=512]
       into a double-buffered PSUM tile [128,1024]; exp on ACT per round
       (strided APs skip the pad columns of the ragged 476-wide q chunk).
  U-trick: wv_q[q(128),65] += P^T[k,q].T @ v_aug[k,65] per (qt, kt) -- the
       free dim is 65 instead of ~500, saving ~70k PE cycles/core vs the
       [65,q] orientation; denominators ride along as column 64. One
       start/stop per u tile: matmul start=True zeroes the whole 2KB PSUM
       zero-region, so per-qt groups would wipe each other.
  normalize: per (hp,e) one DVE reciprocal [128,4] + one stride-0-broadcast
       multiply (q-major: the denominator is a per-partition scalar, so the
       baseline's DRAM broadcast bounce disappears).
  wv_q -> wvT via 4 accumulating PE transposes into one [128,512] PSUM tile
       + one DVE copy per (hp, chunk), then the output projection
       y_partial[1500,1024] = wvT.T @ woT (y stored bf16; host upcasts).

The emission order software-pipelines everything around the in-order PE
stream: a lead-in runs six st-interleaved projection pieces under the x
DMA, then v; attention rounds carry U-work lagged two rounds ahead of the
scores (so nothing waits on a just-issued exp), transposes deferred into
the next hp, and the remaining projection/output-projection pieces woven
in as fillers at the known stall sites (r3 + the two hp-boundary slots).
PSUM budget: st 2x2 + u 2 + pj 2 = 8 banks. PE busy ~176us of ~211us
total; ACT (exp) ~147us.
"""

import os
import numpy as np

N_STATE = 1024
B = 4
S = 1500
F = 512          # features per core (8 heads x 64)
NST = 8          # state k-tiles of 128 (contraction for projections)
NKT = 12         # seq k-tiles of 128 (attention contraction), last = 92
KPAD = 1536      # padded k extent (12*128)
QCH = [(0, 512), (512, 512), (1024, 476)]  # q chunks
VBLK = 520       # 8 heads * 65 cols (64 d + ones) per seq tile in v_sb
SCALE = 0.125    # 1/sqrt(64)
NCORES = 8
NR = 6           # kt rounds per (hp, e, chunk): rounds of 2 kt

_CACHE = {}
LAST_RESULTS = None


def _build():
    import concourse.bass as bass
    import concourse.mybir as mybir
    import concourse.tile as tile
    from concourse import bacc

    f32 = mybir.dt.float32
    bf16 = mybir.dt.bfloat16
    Exp = mybir.ActivationFunctionType.Exp
    Copy = mybir.ActivationFunctionType.Copy
    mdt = bf16

    nc = bacc.Bacc("TRN2", target_bir_lowering=False, debug=False,
                   num_devices=NCORES)

    xT = nc.dram_tensor("xT", [N_STATE, S], mdt, kind="ExternalInput").ap()
    wq = nc.dram_tensor("wq", [F, N_STATE], mdt, kind="ExternalInput").ap()
    wk = nc.dram_tensor("wk", [F, N_STATE], mdt, kind="ExternalInput").ap()
    wv = nc.dram_tensor("wv", [128, NST * F], mdt, kind="ExternalInput").ap()
    wo = nc.dram_tensor("wo", [128, 4 * N_STATE], mdt,
                        kind="ExternalInput").ap()
    bq = nc.dram_tensor("bq", [F], f32, kind="ExternalInput").ap()
    bv = nc.dram_tensor("bv", [F], f32, kind="ExternalInput").ap()
    zinit = nc.dram_tensor("zinit", [4 * (KPAD - S)], mdt,
                           kind="ExternalInput").ap()
    ident = nc.dram_tensor("ident", [128, 128], mdt,
                           kind="ExternalInput").ap()
    y = nc.dram_tensor("y", [S, N_STATE], mdt, kind="ExternalOutput").ap()

    def mm(out, lhsT, rhs, **kw):
        nc.tensor.matmul(out=out, lhsT=lhsT, rhs=rhs, **kw)

    with tile.TileContext(nc) as tc:
        with (
            tc.tile_pool(name="sb", bufs=1) as sb,
            tc.tile_pool(name="sbw", bufs=6) as sbw,
            tc.tile_pool(name="ptp", bufs=16) as ptp,
            tc.tile_pool(name="wvq", bufs=4) as wvqp,
            tc.tile_pool(name="sm", bufs=8) as sm,
            tc.tile_pool(name="ysp", bufs=8) as ysp,
            tc.tile_pool(name="pst", bufs=2, space="PSUM") as pstp,
            tc.tile_pool(name="pu", bufs=2, space="PSUM") as pup,
            tc.tile_pool(name="pj", bufs=2, space="PSUM") as pjp,
        ):
            # ---------------- persistent SBUF ----------------
            xT_sb = sb.tile([128, NST * S], mdt, name="xT_sb", tag="xT")
            qT_sb = sb.tile([128, 4 * S], mdt, name="qT_sb", tag="qT")
            kT_sb = sb.tile([128, 4 * KPAD], mdt, name="kT_sb", tag="kT")
            v_sb = sb.tile([128, NKT * VBLK], mdt, name="v_sb", tag="v")
            wvT_sb = sb.tile([128, 4 * KPAD], mdt, name="wvT_sb", tag="wvT")
            wv_sb = sb.tile([128, NST * F], mdt, name="wv_sb", tag="wvw")
            wo_sb = sb.tile([128, 4 * N_STATE], mdt, name="wo_sb", tag="wow")
            bq_sb = sb.tile([128, 4], f32, name="bq_sb", tag="bq")
            bvb_sb = sb.tile([128, F], f32, name="bvb_sb", tag="bvb")
            id_sb = sb.tile([128, 128], mdt, name="id_sb", tag="id")
            wqsl = [sb.tile([128, NST * 128], mdt, name=f"wqsl{ft}",
                            tag=f"wq{ft}") for ft in range(4)]

            # ---------------- input DMAs (ordered by first use) -----------
            wsl_k0 = sbw.tile([128, NST * 128], mdt, name="wsl_k0",
                              tag="wsl")
            nc.sync.dma_start(out=wsl_k0[:, 0:256], in_=wk[0:128, 0:256])
            nc.sync.dma_start(out=wsl_k0[:, 256:1024],
                              in_=wk[0:128, 256:1024])
            for st in range(NST):
                nc.sync.dma_start(
                    out=xT_sb[:, st * S:(st + 1) * S],
                    in_=xT[st * 128:(st + 1) * 128, :])
            nc.sync.dma_start(out=wqsl[0][:], in_=wq[0:128, :])
            wsl_k1 = sbw.tile([128, NST * 128], mdt, name="wsl_k1",
                              tag="wsl")
            nc.sync.dma_start(out=wsl_k1[:], in_=wk[128:256, :])
            nc.sync.dma_start(out=wqsl[1][:], in_=wq[128:256, :])
            nc.sync.dma_start(out=bq_sb[:],
                              in_=bq.rearrange("(f p) -> p f", p=128))
            nc.sync.dma_start(out=wv_sb[:], in_=wv)
            nc.sync.dma_start(
                out=bvb_sb[:], in_=bv[None, :].to_broadcast((128, F)))
            nc.vector.memset(
                v_sb[:].rearrange("p (t h c) -> p t h c",
                                  t=NKT, h=8)[:, :, :, 64:65], 1.0)
            nc.sync.dma_start(out=id_sb[:], in_=ident)
            nc.sync.dma_start(
                out=kT_sb[:].rearrange("p (f c) -> p f c",
                                       f=4)[:, :, S:KPAD],
                in_=zinit.rearrange("(f c) -> f c", f=4)[None].to_broadcast(
                    (128, 4, KPAD - S)))

            # ---------------- projections ----------------
            def proj_piece(wsl, dst, dstride, biased, ft, qi):
                """One (feature-tile, q-chunk) slice of the q/k projection."""
                q0, qn = QCH[qi]
                pacc = pjp.tile([128, 512], f32, name="pacc", tag="pj")
                for st in range(NST):
                    mm(out=pacc[:, 0:qn],
                       lhsT=wsl[:, st * 128:(st + 1) * 128],
                       rhs=xT_sb[:, st * S + q0:st * S + q0 + qn],
                       start=(st == 0), stop=(st == NST - 1))
                if biased:
                    nc.vector.tensor_scalar_add(
                        out=dst[:, ft * dstride + q0:ft * dstride + q0 + qn],
                        in0=pacc[:, 0:qn], scalar1=bq_sb[:, ft:ft + 1])
                else:
                    nc.vector.tensor_copy(
                        out=dst[:, ft * dstride + q0:ft * dstride + q0 + qn],
                        in_=pacc[:, 0:qn])

            def k_slice(ft):
                wsl = sbw.tile([128, NST * 128], mdt, name="wsl", tag="wsl")
                nc.sync.dma_start(out=wsl[:],
                                  in_=wk[ft * 128:(ft + 1) * 128, :])
                return wsl

            def v_piece(sq):
                """v projection for one seq tile, bv folded in."""
                sn = min(128, S - sq * 128)
                pv = pjp.tile([128, 512], f32, name="pv", tag="pj")
                for st in range(NST):
                    mm(out=pv[0:sn, :],
                       lhsT=xT_sb[:, st * S + sq * 128:
                                  st * S + sq * 128 + sn],
                       rhs=wv_sb[:, st * F:(st + 1) * F],
                       start=(st == 0), stop=(st == NST - 1))
                nc.vector.tensor_add(
                    out=v_sb[0:sn, sq * VBLK:(sq + 1) * VBLK].rearrange(
                        "p (h c) -> p h c", h=8)[:, :, 0:64],
                    in0=pv[0:sn, :].rearrange("p (h c) -> p h c", h=8),
                    in1=bvb_sb[0:sn, :].rearrange("p (h c) -> p h c", h=8))

            def out_piece(sq, ch, quarters=False):
                """Half of the output projection for one seq tile."""
                sn = min(128, S - sq * 128)
                py = pjp.tile([128, 512], f32, name="py", tag="pj")
                nq = 2 if quarters else 1
                for sub in range(nq):
                    c0 = ch * 512 + sub * 256
                    w = 512 // nq
                    for hp in range(4):
                        mm(out=py[0:sn, sub * w:(sub + 1) * w],
                           lhsT=wvT_sb[:, hp * KPAD + sq * 128:
                                       hp * KPAD + sq * 128 + sn],
                           rhs=wo_sb[:, hp * N_STATE + c0:
                                     hp * N_STATE + c0 + w],
                           start=(hp == 0 and sub == 0),
                           stop=(hp == 3 and sub == nq - 1))
                    yt = ysp.tile([128, w], mdt, name="yt", tag="yt")
                    use_act = (quarters and sub == 1) or (sq >= 8 and ch == 1)
                    if use_act:
                        nc.scalar.activation(yt[0:sn, :],
                                             py[0:sn, sub * w:(sub + 1) * w],
                                             Copy)
                    else:
                        nc.vector.tensor_copy(
                            out=yt[0:sn, :],
                            in_=py[0:sn, sub * w:(sub + 1) * w])
                    nc.sync.dma_start(
                        out=y[sq * 128:sq * 128 + sn, c0:c0 + w],
                        in_=yt[0:sn, :])

            # ---------------- filler management ----------------
            fillers = []

            def fill(n):
                for _ in range(min(n, len(fillers))):
                    fillers.pop(0)()

            # ---------------- attention ----------------
            def scores(hp, e, qi, r):
                q0, qn = QCH[qi]
                pb = 64 * e
                st_ps = pstp.tile([128, 1024], f32, name="st_ps", tag="st")
                for j in range(2):
                    kt = 2 * r + j
                    mm(out=st_ps[:, j * 512:j * 512 + qn],
                       lhsT=kT_sb[pb:pb + 64,
                                  hp * KPAD + kt * 128:
                                  hp * KPAD + (kt + 1) * 128],
                       rhs=qT_sb[pb:pb + 64,
                                 hp * S + q0:hp * S + q0 + qn])
                pt = ptp.tile([128, 1024], mdt, name="pt", tag="pt")
                if qn == 512:
                    nc.scalar.activation(pt[:], st_ps[:], Exp, scale=SCALE)
                else:
                    nc.scalar.activation(
                        pt[:].rearrange("p (j q) -> p j q",
                                        j=2)[:, :, 0:qn],
                        st_ps[:].rearrange("p (j q) -> p j q",
                                           j=2)[:, :, 0:qn],
                        Exp, scale=SCALE)
                return pt

            def u_round(hp, e, r, pt, u):
                head = 2 * hp + e
                # start=True zeroes the whole 2KB psum zero-region (bank),
                # so exactly one start/stop for the tile's four qt groups.
                for qt in range(4):
                    for j in range(2):
                        kt = 2 * r + j
                        kn = min(128, S - kt * 128)
                        mm(out=u[:, qt * 65:qt * 65 + 65],
                           lhsT=pt[0:kn, j * 512 + qt * 128:
                                   j * 512 + (qt + 1) * 128],
                           rhs=v_sb[0:kn, kt * VBLK + head * 65:
                                    kt * VBLK + head * 65 + 65],
                           start=(r == 0 and j == 0 and qt == 0),
                           stop=(r == NR - 1 and j == 1 and qt == 3))

            def normalize(hp, e, u, wv_q):
                rc = sm.tile([128, 4], f32, name="rc", tag="rc")
                nc.vector.reciprocal(rc[:], u[:, 64:260:65])
                nc.vector.tensor_mul(
                    wv_q[:].rearrange("p (q d) -> p q d",
                                      q=4)[:, :, (2 * hp + e) * 64:
                                           (2 * hp + e) * 64 + 64],
                    u[:, 0:260].rearrange("p (q c) -> p q c",
                                          q=4)[:, :, 0:64],
                    rc[:].unsqueeze(2).to_broadcast((128, 4, 64)))

            def transposes(hp, qi, wv_q):
                q0, _ = QCH[qi]
                tp = pjp.tile([128, 512], mdt, name="tp", tag="pj")
                for qt in range(4):
                    nc.tensor.matmul(
                        out=tp[:, qt * 128:(qt + 1) * 128],
                        lhsT=wv_q[:, qt * 512 + hp * 128:
                                  qt * 512 + (hp + 1) * 128],
                        rhs=id_sb[:], is_transpose=True,
                        start=(qt == 0), stop=(qt == 3))
                nc.vector.tensor_copy(
                    out=wvT_sb[:, hp * KPAD + q0:hp * KPAD + q0 + 512],
                    in_=tp[:])

            pending = []
            deferred = []

            def flush_pending():
                while pending:
                    h, q, w = pending.pop(0)
                    transposes(h, q, w)

            # ---------------- emission schedule ----------------
            # lead-in: kT(ft0) + qT(ft0, chunk0) + all of v, so hp0/chunk0
            # attention can start and its U rounds have every v tile. The
            # four ft0 pieces run st-interleaved so each arriving x tile
            # feeds ~0.9us of PE work instead of 0.2us.
            lead = [(wsl_k0, kT_sb, KPAD, False, 0, 0),
                    (wqsl[0], qT_sb, S, True, 0, 0),
                    (wsl_k0, kT_sb, KPAD, False, 0, 1),
                    (wsl_k0, kT_sb, KPAD, False, 0, 2),
                    (wsl_k1, kT_sb, KPAD, False, 1, 0),
                    (wqsl[1], qT_sb, S, True, 1, 0)]
            pools = ["pj", "pj", "st", "st", "u", "u"]
            lead_acc = []
            for gi, (wsl, dst, dstride, biased, ft, qi) in enumerate(lead):
                pool = {"pj": pjp, "st": pstp, "u": pup}[pools[gi]]
                lead_acc.append(pool.tile(
                    [128, 512], f32, name=f"lacc{gi}", tag=pools[gi]))
            for st in range(NST):
                for gi, (wsl, dst, dstride, biased, ft, qi) in \
                        enumerate(lead):
                    q0, qn = QCH[qi]
                    mm(out=lead_acc[gi][:, 0:qn],
                       lhsT=wsl[:, st * 128:(st + 1) * 128],
                       rhs=xT_sb[:, st * S + q0:st * S + q0 + qn],
                       start=(st == 0), stop=(st == NST - 1))
            # ACT is idle before attention: alternate the lead copies
            # across ACT/DVE so they drain in parallel (first scores gate
            # on the kc0/qc0 copies).
            Ident = mybir.ActivationFunctionType.Identity
            for gi, (wsl, dst, dstride, biased, ft, qi) in enumerate(lead):
                q0, qn = QCH[qi]
                dsl = dst[:, ft * dstride + q0:ft * dstride + q0 + qn]
                if gi % 2 == 0:
                    if biased:
                        nc.scalar.activation(dsl, lead_acc[gi][:, 0:qn],
                                             Ident,
                                             bias=bq_sb[:, ft:ft + 1])
                    else:
                        nc.scalar.activation(dsl, lead_acc[gi][:, 0:qn],
                                             Copy)
                elif biased:
                    nc.vector.tensor_scalar_add(
                        out=dsl, in0=lead_acc[gi][:, 0:qn],
                        scalar1=bq_sb[:, ft:ft + 1])
                else:
                    nc.vector.tensor_copy(out=dsl,
                                          in_=lead_acc[gi][:, 0:qn])
            for sq in range(NKT):
                v_piece(sq)

            # filler queue: remaining projections in deadline order
            # (kT ft before attention hp=ft; qT chunk qi before chunk qi).
            for qi in range(1, 3):
                fillers.append(
                    lambda qi=qi: proj_piece(wsl_k1, kT_sb, KPAD, False,
                                             1, qi))
            for ft in range(2, 4):
                wsl_k = k_slice(ft)
                nc.sync.dma_start(out=wqsl[ft][:],
                                  in_=wq[ft * 128:(ft + 1) * 128, :])
                for qi in range(3):
                    fillers.append(
                        lambda ft=ft, qi=qi, w=wsl_k: proj_piece(
                            w, kT_sb, KPAD, False, ft, qi))
                fillers.append(
                    lambda ft=ft: proj_piece(wqsl[ft], qT_sb, S, True,
                                             ft, 0))
            nc.sync.dma_start(out=wo_sb[:], in_=wo)
            for qi in range(1, 3):
                for ft in range(4):
                    fillers.append(
                        lambda ft=ft, qi=qi: proj_piece(
                            wqsl[ft], qT_sb, S, True, ft, qi))

            for qi in range(3):
                wv_q = wvqp.tile([128, 4 * 512], mdt, name="wv_q",
                                 tag="wvq")
                for hp in range(4):
                    us = [pup.tile([128, 260], f32, name=f"u{e}", tag="u")
                          for e in range(2)]
                    pts = {}
                    # U rounds lag the scores by TWO rounds and sit BEFORE
                    # them in the in-order PE stream, so the PE never waits
                    # on an exp that was just issued.
                    for r in range(NR):
                        if r >= 2:
                            u_round(hp, 0, r - 2, pts.pop((0, r - 2)),
                                    us[0])
                            u_round(hp, 1, r - 2, pts.pop((1, r - 2)),
                                    us[1])
                        if r == 2:
                            flush_pending()
                        if r == 4:
                            fill(1)
                        pts[(0, r)] = scores(hp, 0, qi, r)
                        pts[(1, r)] = scores(hp, 1, qi, r)
                    u_round(hp, 0, NR - 2, pts.pop((0, NR - 2)), us[0])
                    u_round(hp, 1, NR - 2, pts.pop((1, NR - 2)), us[1])
                    fill(1)
                    u_round(hp, 0, NR - 1, pts.pop((0, NR - 1)), us[0])
                    normalize(hp, 0, us[0], wv_q)
                    u_round(hp, 1, NR - 1, pts.pop((1, NR - 1)), us[1])
                    normalize(hp, 1, us[1], wv_q)
                    if qi == 2 and hp == 3:
                        # last hp: transposes must precede the tail
                        # out-pieces the boundary fills are about to pop
                        pending.append((hp, qi, wv_q))
                        flush_pending()
                        fill(1)
                        fill(1)
                    else:
                        fill(1)
                        fill(1)
                        pending.append((hp, qi, wv_q))
                # out-projection for this chunk becomes filler work for
                # the later chunks' ACT-bound stretches; half of chunk0's
                # pieces are deferred one more chunk to feed chunk2
                for sq in range(4 * qi, 4 * qi + 4):
                    fillers.append(
                        lambda sq=sq: out_piece(sq, 0))
                for sq in range(4 * qi, 4 * qi + 4):
                    item = (lambda sq=sq: out_piece(sq, 1))
                    if qi == 0:
                        deferred.append(item)
                    else:
                        fillers.append(item)
                fillers.extend(deferred)
                deferred = []
            flush_pending()
            while fillers:
                fillers.pop(0)()

    nc.compile()
    return nc


def get_nc():
    if "nc" not in _CACHE:
        _CACHE["nc"] = _build()
    return _CACHE["nc"]


def make_in_maps(x, Wq, bq, Wk, Wv, bv, Wo, bo):
    import ml_dtypes
    bf = ml_dtypes.bfloat16
    x = np.asarray(x, dtype=np.float32)
    Wq = np.asarray(Wq, dtype=np.float32)
    Wk = np.asarray(Wk, dtype=np.float32)
    Wv = np.asarray(Wv, dtype=np.float32)
    Wo = np.asarray(Wo, dtype=np.float32)
    bq = np.asarray(bq, dtype=np.float32)
    bv = np.asarray(bv, dtype=np.float32)
    ident = np.eye(128, dtype=bf)
    in_maps = []
    for c in range(NCORES):
        b, h = divmod(c, 2)
        sl = slice(h * F, (h + 1) * F)
        wq_t = Wq[sl, :].T.reshape(8, 128, 4, 128)
        wk_t = Wk[sl, :].T.reshape(8, 128, 4, 128)
        wv_t = Wv[sl, :].T.reshape(8, 128, 512)
        wo_t = Wo[:, sl].T.reshape(4, 128, 1024)
        in_maps.append(dict(
            xT=np.ascontiguousarray(x[b].T).astype(bf),
            wq=np.ascontiguousarray(
                wq_t.transpose(2, 1, 0, 3).reshape(512, 1024)).astype(bf),
            wk=np.ascontiguousarray(
                wk_t.transpose(2, 1, 0, 3).reshape(512, 1024)).astype(bf),
            wv=np.ascontiguousarray(
                wv_t.transpose(1, 0, 2).reshape(128, 4096)).astype(bf),
            wo=np.ascontiguousarray(
                wo_t.transpose(1, 0, 2).reshape(128, 4096)).astype(bf),
            bq=np.ascontiguousarray(bq[sl]),
            bv=np.ascontiguousarray(bv[sl]),
            zinit=np.zeros(4 * (KPAD - S), dtype=bf),
            ident=ident,
        ))
    return in_maps


def kernel(x, Wq, bq, Wk, Wv, bv, Wo, bo):
    global LAST_RESULTS
    from concourse.bass_utils import run_bass_kernel_spmd

    # An ambient BASS_TRACE=1 makes run_bass_kernel_spmd fetch the axon NTFF
    # profile hook; in containers where antenv.axon_hooks is absent that
    # import crashes. Disable tracing for this call when the hook is missing.
    try:
        import antenv.axon_hooks  # noqa: F401
    except ImportError:
        os.environ["BASS_NEVER_TRACE"] = "1"

    nc = get_nc()
    in_maps = make_in_maps(x, Wq, bq, Wk, Wv, bv, Wo, bo)
    res = run_bass_kernel_spmd(nc, in_maps, list(range(NCORES)))
    LAST_RESULTS = res
    bo32 = np.asarray(bo, dtype=np.float32)
    out = np.stack([res.results[2 * b]["y"].astype(np.float32)
                    + res.results[2 * b + 1]["y"].astype(np.float32)
                    + bo32[None, :] for b in range(B)])
    return out.astype(np.float32)


# revision 46
# speedup vs baseline: 1.0046x; 1.0015x over previous
"""Multi-head attention (B=4, S=1500, D=1024, H=16) on 8 TRN2 NeuronCores.

Sharding: (batch, head-half) -> core c = 2*b + h; each core computes the
full attention for batch b, heads h*8..h*8+7, plus its partial contribution
to the output projection (contraction over its 512 features). Host sums the
two partials per batch and stacks.

All matmul operands are bf16 (measured end-to-end rel err ~5e-3 vs the 2e-2
gate). Per-core layout / schedule:
  xT   [1024,1500]  host-pretransposed x[b].T (bf16)
  qT/kT = W^T.T @ xT -> feature-major [4x128, 1500/1536] per 128-feat tile
  v    [1500, 8*65] natural, 65th column per head = ones (softmax denom);
       bv folded into v (softmax rows sum to 1 -> wv + bv comes out).
  scores: per (head-pair hp, head e, 2-kt round r): S^T tiles [128k, q<=512]
       into a double-buffered PSUM tile [128,1024]; exp on ACT per round
       (strided APs skip the pad columns of the ragged 476-wide q chunk).
  U-trick: wv_q[q(128),65] += P^T[k,q].T @ v_aug[k,65] per (qt, kt) -- the
       free dim is 65 instead of ~500, saving ~70k PE cycles/core vs the
       [65,q] orientation; denominators ride along as column 64. One
       start/stop per u tile: matmul start=True zeroes the whole 2KB PSUM
       zero-region, so per-qt groups would wipe each other.
  normalize: per (hp,e) one DVE reciprocal [128,4] + one stride-0-broadcast
       multiply (q-major: the denominator is a per-partition scalar, so the
       baseline's DRAM broadcast bounce disappears).
  wv_q -> wvT via 4 accumulating PE transposes into one [128,512] PSUM tile
       + one DVE copy per (hp, chunk), then the output projection
       y_partial[1500,1024] = wvT.T @ woT (y stored bf16; host upcasts).

The emission order software-pipelines everything around the in-order PE
stream: a lead-in runs six st-interleaved projection pieces under the x
DMA, then v; attention rounds carry U-work lagged two rounds ahead of the
scores (so nothing waits on a just-issued exp), transposes deferred into
the next hp, and the remaining projection/output-projection pieces woven
in as fillers at the known stall sites (r3 + the two hp-boundary slots).
PSUM budget: st 2x2 + u 2 + pj 2 = 8 banks. PE busy ~176us of ~211us
total; ACT (exp) ~147us.
"""

import os
import numpy as np

N_STATE = 1024
B = 4
S = 1500
F = 512          # features per core (8 heads x 64)
NST = 8          # state k-tiles of 128 (contraction for projections)
NKT = 12         # seq k-tiles of 128 (attention contraction), last = 92
KPAD = 1536      # padded k extent (12*128)
QCH = [(0, 512), (512, 512), (1024, 476)]  # q chunks
VBLK = 520       # 8 heads * 65 cols (64 d + ones) per seq tile in v_sb
SCALE = 0.125    # 1/sqrt(64)
NCORES = 8
NR = 6           # kt rounds per (hp, e, chunk): rounds of 2 kt

_CACHE = {}
LAST_RESULTS = None


def _build():
    import concourse.bass as bass
    import concourse.mybir as mybir
    import concourse.tile as tile
    from concourse import bacc

    f32 = mybir.dt.float32
    bf16 = mybir.dt.bfloat16
    Exp = mybir.ActivationFunctionType.Exp
    Copy = mybir.ActivationFunctionType.Copy
    mdt = bf16

    nc = bacc.Bacc("TRN2", target_bir_lowering=False, debug=False,
                   num_devices=NCORES)

    xT = nc.dram_tensor("xT", [N_STATE, S], mdt, kind="ExternalInput").ap()
    wq = nc.dram_tensor("wq", [F, N_STATE], mdt, kind="ExternalInput").ap()
    wk = nc.dram_tensor("wk", [F, N_STATE], mdt, kind="ExternalInput").ap()
    wv = nc.dram_tensor("wv", [128, NST * F], mdt, kind="ExternalInput").ap()
    wo = nc.dram_tensor("wo", [128, 4 * N_STATE], mdt,
                        kind="ExternalInput").ap()
    bq = nc.dram_tensor("bq", [F], f32, kind="ExternalInput").ap()
    bv = nc.dram_tensor("bv", [F], f32, kind="ExternalInput").ap()
    zinit = nc.dram_tensor("zinit", [4 * (KPAD - S)], mdt,
                           kind="ExternalInput").ap()
    ident = nc.dram_tensor("ident", [128, 128], mdt,
                           kind="ExternalInput").ap()
    y = nc.dram_tensor("y", [S, N_STATE], mdt, kind="ExternalOutput").ap()

    def mm(out, lhsT, rhs, **kw):
        nc.tensor.matmul(out=out, lhsT=lhsT, rhs=rhs, **kw)

    with tile.TileContext(nc) as tc:
        with (
            tc.tile_pool(name="sb", bufs=1) as sb,
            tc.tile_pool(name="sbw", bufs=6) as sbw,
            tc.tile_pool(name="ptp", bufs=16) as ptp,
            tc.tile_pool(name="wvq", bufs=4) as wvqp,
            tc.tile_pool(name="sm", bufs=8) as sm,
            tc.tile_pool(name="ysp", bufs=8) as ysp,
            tc.tile_pool(name="pst", bufs=2, space="PSUM") as pstp,
            tc.tile_pool(name="pu", bufs=2, space="PSUM") as pup,
            tc.tile_pool(name="pj", bufs=2, space="PSUM") as pjp,
        ):
            # ---------------- persistent SBUF ----------------
            xT_sb = sb.tile([128, NST * S], mdt, name="xT_sb", tag="xT")
            qT_sb = sb.tile([128, 4 * S], mdt, name="qT_sb", tag="qT")
            kT_sb = sb.tile([128, 4 * KPAD], mdt, name="kT_sb", tag="kT")
            v_sb = sb.tile([128, NKT * VBLK], mdt, name="v_sb", tag="v")
            wvT_sb = sb.tile([128, 4 * KPAD], mdt, name="wvT_sb", tag="wvT")
            wv_sb = sb.tile([128, NST * F], mdt, name="wv_sb", tag="wvw")
            wo_sb = sb.tile([128, 4 * N_STATE], mdt, name="wo_sb", tag="wow")
            bq_sb = sb.tile([128, 4], f32, name="bq_sb", tag="bq")
            bvb_sb = sb.tile([128, F], f32, name="bvb_sb", tag="bvb")
            id_sb = sb.tile([128, 128], mdt, name="id_sb", tag="id")
            wqsl = [sb.tile([128, NST * 128], mdt, name=f"wqsl{ft}",
                            tag=f"wq{ft}") for ft in range(4)]

            # ---------------- input DMAs (ordered by first use) -----------
            wsl_k0 = sbw.tile([128, NST * 128], mdt, name="wsl_k0",
                              tag="wsl")
            nc.sync.dma_start(out=wsl_k0[:, 0:256], in_=wk[0:128, 0:256])
            nc.sync.dma_start(out=wsl_k0[:, 256:1024],
                              in_=wk[0:128, 256:1024])
            for st in range(NST):
                nc.sync.dma_start(
                    out=xT_sb[:, st * S:(st + 1) * S],
                    in_=xT[st * 128:(st + 1) * 128, :])
            nc.sync.dma_start(out=wqsl[0][:], in_=wq[0:128, :])
            wsl_k1 = sbw.tile([128, NST * 128], mdt, name="wsl_k1",
                              tag="wsl")
            nc.sync.dma_start(out=wsl_k1[:], in_=wk[128:256, :])
            nc.sync.dma_start(out=wqsl[1][:], in_=wq[128:256, :])
            nc.sync.dma_start(out=bq_sb[:],
                              in_=bq.rearrange("(f p) -> p f", p=128))
            nc.sync.dma_start(out=wv_sb[:], in_=wv)
            nc.sync.dma_start(
                out=bvb_sb[:], in_=bv[None, :].to_broadcast((128, F)))
            nc.vector.memset(
                v_sb[:].rearrange("p (t h c) -> p t h c",
                                  t=NKT, h=8)[:, :, :, 64:65], 1.0)
            nc.sync.dma_start(out=id_sb[:], in_=ident)
            nc.sync.dma_start(
                out=kT_sb[:].rearrange("p (f c) -> p f c",
                                       f=4)[:, :, S:KPAD],
                in_=zinit.rearrange("(f c) -> f c", f=4)[None].to_broadcast(
                    (128, 4, KPAD - S)))

            # ---------------- projections ----------------
            def proj_piece(wsl, dst, dstride, biased, ft, qi):
                """One (feature-tile, q-chunk) slice of the q/k projection."""
                q0, qn = QCH[qi]
                pacc = pjp.tile([128, 512], f32, name="pacc", tag="pj")
                for st in range(NST):
                    mm(out=pacc[:, 0:qn],
                       lhsT=wsl[:, st * 128:(st + 1) * 128],
                       rhs=xT_sb[:, st * S + q0:st * S + q0 + qn],
                       start=(st == 0), stop=(st == NST - 1))
                if biased:
                    nc.vector.tensor_scalar_add(
                        out=dst[:, ft * dstride + q0:ft * dstride + q0 + qn],
                        in0=pacc[:, 0:qn], scalar1=bq_sb[:, ft:ft + 1])
                else:
                    nc.vector.tensor_copy(
                        out=dst[:, ft * dstride + q0:ft * dstride + q0 + qn],
                        in_=pacc[:, 0:qn])

            def k_slice(ft):
                wsl = sbw.tile([128, NST * 128], mdt, name="wsl", tag="wsl")
                nc.sync.dma_start(out=wsl[:],
                                  in_=wk[ft * 128:(ft + 1) * 128, :])
                return wsl

            def v_piece(sq):
                """v projection for one seq tile, bv folded in."""
                sn = min(128, S - sq * 128)
                pv = pjp.tile([128, 512], f32, name="pv", tag="pj")
                for st in range(NST):
                    mm(out=pv[0:sn, :],
                       lhsT=xT_sb[:, st * S + sq * 128:
                                  st * S + sq * 128 + sn],
                       rhs=wv_sb[:, st * F:(st + 1) * F],
                       start=(st == 0), stop=(st == NST - 1))
                nc.vector.tensor_add(
                    out=v_sb[0:sn, sq * VBLK:(sq + 1) * VBLK].rearrange(
                        "p (h c) -> p h c", h=8)[:, :, 0:64],
                    in0=pv[0:sn, :].rearrange("p (h c) -> p h c", h=8),
                    in1=bvb_sb[0:sn, :].rearrange("p (h c) -> p h c", h=8))

            def out_piece(sq, ch, quarters=False):
                """Half of the output projection for one seq tile."""
                sn = min(128, S - sq * 128)
                py = pjp.tile([128, 512], f32, name="py", tag="pj")
                nq = 2 if quarters else 1
                for sub in range(nq):
                    c0 = ch * 512 + sub * 256
                    w = 512 // nq
                    for hp in range(4):
                        mm(out=py[0:sn, sub * w:(sub + 1) * w],
                           lhsT=wvT_sb[:, hp * KPAD + sq * 128:
                                       hp * KPAD + sq * 128 + sn],
                           rhs=wo_sb[:, hp * N_STATE + c0:
                                     hp * N_STATE + c0 + w],
                           start=(hp == 0 and sub == 0),
                           stop=(hp == 3 and sub == nq - 1))
                    yt = ysp.tile([128, w], mdt, name="yt", tag="yt")
                    use_act = (quarters and sub == 1) or (sq >= 8 and ch == 1)
                    if use_act:
                        nc.scalar.activation(yt[0:sn, :],
                                             py[0:sn, sub * w:(sub + 1) * w],
                                             Copy)
                    else:
                        nc.vector.tensor_copy(
                            out=yt[0:sn, :],
                            in_=py[0:sn, sub * w:(sub + 1) * w])
                    nc.sync.dma_start(
                        out=y[sq * 128:sq * 128 + sn, c0:c0 + w],
                        in_=yt[0:sn, :])

            # ---------------- filler management ----------------
            fillers = []

            def fill(n):
                for _ in range(min(n, len(fillers))):
                    fillers.pop(0)()

            # ---------------- attention ----------------
            def scores(hp, e, qi, r):
                q0, qn = QCH[qi]
                pb = 64 * e
                st_ps = pstp.tile([128, 1024], f32, name="st_ps", tag="st")
                for j in range(2):
                    kt = 2 * r + j
                    mm(out=st_ps[:, j * 512:j * 512 + qn],
                       lhsT=kT_sb[pb:pb + 64,
                                  hp * KPAD + kt * 128:
                                  hp * KPAD + (kt + 1) * 128],
                       rhs=qT_sb[pb:pb + 64,
                                 hp * S + q0:hp * S + q0 + qn])
                pt = ptp.tile([128, 1024], mdt, name="pt", tag="pt")
                if qn == 512:
                    nc.scalar.activation(pt[:], st_ps[:], Exp, scale=SCALE)
                else:
                    nc.scalar.activation(
                        pt[:].rearrange("p (j q) -> p j q",
                                        j=2)[:, :, 0:qn],
                        st_ps[:].rearrange("p (j q) -> p j q",
                                           j=2)[:, :, 0:qn],
                        Exp, scale=SCALE)
                return pt

            def u_round(hp, e, r, pt, u):
                head = 2 * hp + e
                # start=True zeroes the whole 2KB psum zero-region (bank),
                # so exactly one start/stop for the tile's four qt groups.
                for qt in range(4):
                    for j in range(2):
                        kt = 2 * r + j
                        kn = min(128, S - kt * 128)
                        mm(out=u[:, qt * 65:qt * 65 + 65],
                           lhsT=pt[0:kn, j * 512 + qt * 128:
                                   j * 512 + (qt + 1) * 128],
                           rhs=v_sb[0:kn, kt * VBLK + head * 65:
                                    kt * VBLK + head * 65 + 65],
                           start=(r == 0 and j == 0 and qt == 0),
                           stop=(r == NR - 1 and j == 1 and qt == 3))

            def normalize(hp, e, u, wv_q):
                rc = sm.tile([128, 4], f32, name="rc", tag="rc")
                nc.vector.reciprocal(rc[:], u[:, 64:260:65])
                nc.vector.tensor_mul(
                    wv_q[:].rearrange("p (q d) -> p q d",
                                      q=4)[:, :, (2 * hp + e) * 64:
                                           (2 * hp + e) * 64 + 64],
                    u[:, 0:260].rearrange("p (q c) -> p q c",
                                          q=4)[:, :, 0:64],
                    rc[:].unsqueeze(2).to_broadcast((128, 4, 64)))

            def transposes(hp, qi, wv_q):
                q0, _ = QCH[qi]
                tp = pjp.tile([128, 512], mdt, name="tp", tag="pj")
                for qt in range(4):
                    nc.tensor.matmul(
                        out=tp[:, qt * 128:(qt + 1) * 128],
                        lhsT=wv_q[:, qt * 512 + hp * 128:
                                  qt * 512 + (hp + 1) * 128],
                        rhs=id_sb[:], is_transpose=True,
                        start=(qt == 0), stop=(qt == 3))
                nc.vector.tensor_copy(
                    out=wvT_sb[:, hp * KPAD + q0:hp * KPAD + q0 + 512],
                    in_=tp[:])

            pending = []
            deferred = []

            def flush_pending():
                while pending:
                    h, q, w = pending.pop(0)
                    transposes(h, q, w)

            # ---------------- emission schedule ----------------
            # lead-in: kT(ft0) + qT(ft0, chunk0) + all of v, so hp0/chunk0
            # attention can start and its U rounds have every v tile. The
            # four ft0 pieces run st-interleaved so each arriving x tile
            # feeds ~0.9us of PE work instead of 0.2us.
            lead = [(wsl_k0, kT_sb, KPAD, False, 0, 0),
                    (wqsl[0], qT_sb, S, True, 0, 0),
                    (wsl_k0, kT_sb, KPAD, False, 0, 1),
                    (wsl_k0, kT_sb, KPAD, False, 0, 2),
                    (wsl_k1, kT_sb, KPAD, False, 1, 0),
                    (wqsl[1], qT_sb, S, True, 1, 0)]
            pools = ["pj", "pj", "st", "st", "u", "u"]
            lead_acc = []
            for gi, (wsl, dst, dstride, biased, ft, qi) in enumerate(lead):
                pool = {"pj": pjp, "st": pstp, "u": pup}[pools[gi]]
                lead_acc.append(pool.tile(
                    [128, 512], f32, name=f"lacc{gi}", tag=pools[gi]))
            for st in range(NST):
                for gi, (wsl, dst, dstride, biased, ft, qi) in \
                        enumerate(lead):
                    q0, qn = QCH[qi]
                    mm(out=lead_acc[gi][:, 0:qn],
                       lhsT=wsl[:, st * 128:(st + 1) * 128],
                       rhs=xT_sb[:, st * S + q0:st * S + q0 + qn],
                       start=(st == 0), stop=(st == NST - 1))
            # ACT is idle before attention: alternate the lead copies
            # across ACT/DVE so they drain in parallel (first scores gate
            # on the kc0/qc0 copies).
            Ident = mybir.ActivationFunctionType.Identity
            for gi, (wsl, dst, dstride, biased, ft, qi) in enumerate(lead):
                q0, qn = QCH[qi]
                dsl = dst[:, ft * dstride + q0:ft * dstride + q0 + qn]
                if gi % 2 == 0:
                    if biased:
                        nc.scalar.activation(dsl, lead_acc[gi][:, 0:qn],
                                             Ident,
                                             bias=bq_sb[:, ft:ft + 1])
                    else:
                        nc.scalar.activation(dsl, lead_acc[gi][:, 0:qn],
                                             Copy)
                elif biased:
                    nc.vector.tensor_scalar_add(
                        out=dsl, in0=lead_acc[gi][:, 0:qn],
                        scalar1=bq_sb[:, ft:ft + 1])
                else:
                    nc.vector.tensor_copy(out=dsl,
                                          in_=lead_acc[gi][:, 0:qn])
            for sq in range(NKT):
                v_piece(sq)

            # filler queue: remaining projections in deadline order
            # (kT ft before attention hp=ft; qT chunk qi before chunk qi).
            for qi in range(1, 3):
                fillers.append(
                    lambda qi=qi: proj_piece(wsl_k1, kT_sb, KPAD, False,
                                             1, qi))
            for ft in range(2, 4):
                wsl_k = k_slice(ft)
                nc.sync.dma_start(out=wqsl[ft][:],
                                  in_=wq[ft * 128:(ft + 1) * 128, :])
                for qi in range(3):
                    fillers.append(
                        lambda ft=ft, qi=qi, w=wsl_k: proj_piece(
                            w, kT_sb, KPAD, False, ft, qi))
                fillers.append(
                    lambda ft=ft: proj_piece(wqsl[ft], qT_sb, S, True,
                                             ft, 0))
            nc.sync.dma_start(out=wo_sb[:], in_=wo)
            for qi in range(1, 3):
                for ft in range(4):
                    fillers.append(
                        lambda ft=ft, qi=qi: proj_piece(
                            wqsl[ft], qT_sb, S, True, ft, qi))

            carry = {}
            for qi in range(3):
                wv_q = wvqp.tile([128, 4 * 512], mdt, name="wv_q",
                                 tag="wvq")
                for hp in range(4):
                    us = [pup.tile([128, 260], f32, name=f"u{e}", tag="u")
                          for e in range(2)]
                    pts = carry
                    carry = {}
                    # U rounds lag the scores by TWO rounds and sit BEFORE
                    # them in the in-order PE stream, so the PE never waits
                    # on an exp that was just issued.
                    for r in range(NR):
                        if r >= 2:
                            u_round(hp, 0, r - 2, pts.pop((0, r - 2)),
                                    us[0])
                            u_round(hp, 1, r - 2, pts.pop((1, r - 2)),
                                    us[1])
                        if r == 2:
                            flush_pending()
                        if r == 4:
                            fill(1)
                        if (0, r) not in pts:
                            pts[(0, r)] = scores(hp, 0, qi, r)
                            pts[(1, r)] = scores(hp, 1, qi, r)
                    u_round(hp, 0, NR - 2, pts.pop((0, NR - 2)), us[0])
                    u_round(hp, 1, NR - 2, pts.pop((1, NR - 2)), us[1])
                    if hp < 3:
                        carry[(0, 0)] = scores(hp + 1, 0, qi, 0)
                        carry[(1, 0)] = scores(hp + 1, 1, qi, 0)
                    fill(1)
                    u_round(hp, 0, NR - 1, pts.pop((0, NR - 1)), us[0])
                    normalize(hp, 0, us[0], wv_q)
                    u_round(hp, 1, NR - 1, pts.pop((1, NR - 1)), us[1])
                    normalize(hp, 1, us[1], wv_q)
                    if qi == 2 and hp == 3:
                        # last hp: transposes must precede the tail
                        # out-pieces the boundary fills are about to pop
                        pending.append((hp, qi, wv_q))
                        flush_pending()
                        fill(1)
                        fill(1)
                    else:
                        fill(1)
                        fill(1)
                        pending.append((hp, qi, wv_q))
                # out-projection for this chunk becomes filler work for
                # the later chunks' ACT-bound stretches; half of chunk0's
                # pieces are deferred one more chunk to feed chunk2
                for sq in range(4 * qi, 4 * qi + 4):
                    fillers.append(
                        lambda sq=sq: out_piece(sq, 0))
                for sq in range(4 * qi, 4 * qi + 4):
                    item = (lambda sq=sq: out_piece(sq, 1))
                    if qi == 0:
                        deferred.append(item)
                    else:
                        fillers.append(item)
                fillers.extend(deferred)
                deferred = []
            flush_pending()
            while fillers:
                fillers.pop(0)()

    nc.compile()
    return nc


def get_nc():
    if "nc" not in _CACHE:
        _CACHE["nc"] = _build()
    return _CACHE["nc"]


def make_in_maps(x, Wq, bq, Wk, Wv, bv, Wo, bo):
    import ml_dtypes
    bf = ml_dtypes.bfloat16
    x = np.asarray(x, dtype=np.float32)
    Wq = np.asarray(Wq, dtype=np.float32)
    Wk = np.asarray(Wk, dtype=np.float32)
    Wv = np.asarray(Wv, dtype=np.float32)
    Wo = np.asarray(Wo, dtype=np.float32)
    bq = np.asarray(bq, dtype=np.float32)
    bv = np.asarray(bv, dtype=np.float32)
    ident = np.eye(128, dtype=bf)
    in_maps = []
    for c in range(NCORES):
        b, h = divmod(c, 2)
        sl = slice(h * F, (h + 1) * F)
        wq_t = Wq[sl, :].T.reshape(8, 128, 4, 128)
        wk_t = Wk[sl, :].T.reshape(8, 128, 4, 128)
        wv_t = Wv[sl, :].T.reshape(8, 128, 512)
        wo_t = Wo[:, sl].T.reshape(4, 128, 1024)
        in_maps.append(dict(
            xT=np.ascontiguousarray(x[b].T).astype(bf),
            wq=np.ascontiguousarray(
                wq_t.transpose(2, 1, 0, 3).reshape(512, 1024)).astype(bf),
            wk=np.ascontiguousarray(
                wk_t.transpose(2, 1, 0, 3).reshape(512, 1024)).astype(bf),
            wv=np.ascontiguousarray(
                wv_t.transpose(1, 0, 2).reshape(128, 4096)).astype(bf),
            wo=np.ascontiguousarray(
                wo_t.transpose(1, 0, 2).reshape(128, 4096)).astype(bf),
            bq=np.ascontiguousarray(bq[sl]),
            bv=np.ascontiguousarray(bv[sl]),
            zinit=np.zeros(4 * (KPAD - S), dtype=bf),
            ident=ident,
        ))
    return in_maps


def kernel(x, Wq, bq, Wk, Wv, bv, Wo, bo):
    global LAST_RESULTS
    from concourse.bass_utils import run_bass_kernel_spmd

    # An ambient BASS_TRACE=1 makes run_bass_kernel_spmd fetch the axon NTFF
    # profile hook; in containers where antenv.axon_hooks is absent that
    # import crashes. Disable tracing for this call when the hook is missing.
    try:
        import antenv.axon_hooks  # noqa: F401
    except ImportError:
        os.environ["BASS_NEVER_TRACE"] = "1"

    nc = get_nc()
    in_maps = make_in_maps(x, Wq, bq, Wk, Wv, bv, Wo, bo)
    res = run_bass_kernel_spmd(nc, in_maps, list(range(NCORES)))
    LAST_RESULTS = res
    bo32 = np.asarray(bo, dtype=np.float32)
    out = np.stack([res.results[2 * b]["y"].astype(np.float32)
                    + res.results[2 * b + 1]["y"].astype(np.float32)
                    + bo32[None, :] for b in range(B)])
    return out.astype(np.float32)
